# revision 5
# baseline (speedup 1.0000x reference)
# Multi-head attention (B=4, S=1024, H=16, D=64) on 8 trn2 NeuronCores.
#
# Sharding: core c handles batch b=c//2 and heads [8*(c%2), 8*(c%2)+8).
# Each core computes Q/K/V projections for its 512 head-dims over its
# batch's full sequence, per-head attention (scores^T layout: [k, q] so
# softmax sums come from a ones-column packed into the ctx matmul), and
# a partial output projection; the host sums the two half-head partials
# per batch and view-transposes the [k, q] attention back to [q, k].
#
# All device compute is bf16 on the PE with f32 PSUM accumulation; the
# attention output is written bf16 and upcast on the host.

import os
import sys

for _p in ("/opt/trn_rl_repo", "/root/.axon_site/_ro/trn_rl_repo"):
    if os.path.isdir(_p) and _p not in sys.path:
        sys.path.append(_p)

import numpy as np
import ml_dtypes

B, S, H, D = 4, 1024, 16, 64
DM = 1024      # model dim
NH = 8         # heads per core
DH = NH * D    # 512 head-dims per core
P = 128
KT = DM // P   # 8 contraction / seq tiles
OT = DH // P   # 4 out-dim tiles per core
NCORES = 8
BF16NP = ml_dtypes.bfloat16

_CACHE = {}


def _build_module():
    import concourse.tile as tile
    from concourse import bacc, mybir

    f32 = mybir.dt.float32
    bf16 = mybir.dt.bfloat16
    Exp = mybir.ActivationFunctionType.Exp
    Ln = mybir.ActivationFunctionType.Ln

    nc = bacc.Bacc(
        "TRN2", target_bir_lowering=False, debug=False, num_devices=NCORES
    )

    qT = nc.dram_tensor("qT", [DM, S], bf16, kind="ExternalInput").ap()
    kT = nc.dram_tensor("kT", [DM, S], bf16, kind="ExternalInput").ap()
    vT = nc.dram_tensor("vT", [DM, S], bf16, kind="ExternalInput").ap()
    wqT = nc.dram_tensor("wqT", [DM, DH], bf16, kind="ExternalInput").ap()
    wkT = nc.dram_tensor("wkT", [DM, DH], bf16, kind="ExternalInput").ap()
    wvT = nc.dram_tensor("wvT", [DM, DH], bf16, kind="ExternalInput").ap()
    woT = nc.dram_tensor("woT", [DH, DM], bf16, kind="ExternalInput").ap()
    attnT = nc.dram_tensor("attnT", [NH, S, S], bf16, kind="ExternalOutput").ap()
    outT = nc.dram_tensor("outT", [DM, S], f32, kind="ExternalOutput").ap()

    with tile.TileContext(nc) as tc:
        with (
            tc.tile_pool(name="singles", bufs=1) as singles,
            tc.tile_pool(name="exps", bufs=2) as exps,
            tc.tile_pool(name="small", bufs=2) as small,
            tc.tile_pool(name="outsb", bufs=3) as outsb,
            tc.tile_pool(name="ps", bufs=2, space="PSUM") as psum,
            tc.tile_pool(name="ctxp", bufs=2, space="PSUM") as ctxpsum,
        ):
            # ---- stage inputs in SBUF ----
            qT_sb = singles.tile([P, KT, S], bf16)
            kT_sb = singles.tile([P, KT, S], bf16)
            vT_sb = singles.tile([P, KT, S], bf16)
            nc.sync.dma_start(out=qT_sb, in_=qT.rearrange("(t p) s -> p t s", p=P))
            nc.sync.dma_start(out=kT_sb, in_=kT.rearrange("(t p) s -> p t s", p=P))
            nc.sync.dma_start(out=vT_sb, in_=vT.rearrange("(t p) s -> p t s", p=P))
            wq_sb = singles.tile([P, KT, DH], bf16)
            wk_sb = singles.tile([P, KT, DH], bf16)
            wv_sb = singles.tile([P, KT, DH], bf16)
            nc.sync.dma_start(out=wq_sb, in_=wqT.rearrange("(t p) d -> p t d", p=P))
            nc.sync.dma_start(out=wk_sb, in_=wkT.rearrange("(t p) d -> p t d", p=P))
            nc.sync.dma_start(out=wv_sb, in_=wvT.rearrange("(t p) d -> p t d", p=P))
            wo_sb = singles.tile([P, OT, DM], bf16)
            nc.sync.dma_start(out=wo_sb, in_=woT.rearrange("(t p) d -> p t d", p=P))
            # lhsT for the normalizer broadcast; lives at base partition 64
            # to match the reciprocal row (PE wants equal base partitions)
            ones_sb = singles.tile([D + 1, P], f32)
            nc.vector.memset(ones_sb[D : D + 1, :], 1.0)

            # QpT/KpT: [part = head-dim within tile, ot, seq]; head h lives on
            # partitions 64*(h%2).. of tile h//2.
            qp_sb = singles.tile([P, OT, S], bf16)
            kp_sb = singles.tile([P, OT, S], bf16)
            # Vp: [part = seq within tile, seq-tile, head, d + ones column]
            vp_sb = singles.tile([P, KT, NH, D + 1], bf16)
            # normalized ctx^T stacked: [part = head-dim within tile, ct, q]
            ctxn_sb = singles.tile([P, OT, S], bf16)

            # ---- Q/K projections: QpT = wqT.T @ qT ----
            for w_sb, x_sb, dst in ((wq_sb, qT_sb, qp_sb), (wk_sb, kT_sb, kp_sb)):
                for ot in range(OT):
                    for nch in range(2):
                        ps = psum.tile([P, S], f32, tag="ps")
                        pslice = ps[:, :512]
                        for kt in range(KT):
                            nc.tensor.matmul(
                                pslice,
                                lhsT=w_sb[:, kt, ot * 128 : (ot + 1) * 128],
                                rhs=x_sb[:, kt, nch * 512 : (nch + 1) * 512],
                                start=(kt == 0),
                                stop=(kt == KT - 1),
                            )
                        nc.vector.tensor_copy(
                            out=dst[:, ot, nch * 512 : (nch + 1) * 512], in_=pslice
                        )

            # ---- V projection: Vp = vT.T.T ... Vp[seq, dh] = (vT.T) @ wvT ----
            nc.vector.memset(vp_sb[:, :, :, D], 1.0)
            for mt in range(KT):
                ps = psum.tile([P, S], f32, tag="ps")
                pslice = ps[:, :512]
                for kt in range(KT):
                    nc.tensor.matmul(
                        pslice,
                        lhsT=vT_sb[:, kt, mt * 128 : (mt + 1) * 128],
                        rhs=wv_sb[:, kt, :],
                        start=(kt == 0),
                        stop=(kt == KT - 1),
                    )
                nc.vector.tensor_copy(
                    out=vp_sb[:, mt, :, 0:D],
                    in_=pslice.rearrange("p (h d) -> p h d", h=NH),
                )

            # ---- attention, one head at a time ----
            for h in range(NH):
                hp = 64 * (h % 2)
                ot = h // 2
                expT = exps.tile([P, KT, S], bf16, tag="expT")
                ctx = ctxpsum.tile([P, S], f32, tag="ctx")
                for kt in range(KT):
                    sc = psum.tile([P, S], f32, tag="ps")
                    for nch in range(2):
                        nc.tensor.matmul(
                            sc[:, nch * 512 : (nch + 1) * 512],
                            lhsT=kp_sb[hp : hp + 64, ot, kt * 128 : (kt + 1) * 128],
                            rhs=qp_sb[hp : hp + 64, ot, nch * 512 : (nch + 1) * 512],
                            start=True,
                            stop=True,
                        )
                    # exp(scores/8); bf16 out feeds both the ctx matmul and,
                    # after normalization, the attn output
                    nc.scalar.activation(
                        out=expT[:, kt, :], in_=sc[:], func=Exp, scale=1.0 / 8.0
                    )
                    for nch in range(2):
                        nc.tensor.matmul(
                            ctx[0 : D + 1, nch * 512 : (nch + 1) * 512],
                            lhsT=vp_sb[:, kt, h, :],
                            rhs=expT[:, kt, nch * 512 : (nch + 1) * 512],
                            start=(kt == 0),
                            stop=(kt == KT - 1),
                        )
                # softmax normalizer: row D of ctx holds sum_k exp;
                # 1/s = exp(-ln(s)) on ScalarE (custom DVE recip is broken
                # on HW at base partition 64)
                lnt = small.tile([D + 1, S], f32, tag="lnt")
                nc.scalar.activation(
                    out=lnt[D : D + 1, :], in_=ctx[D : D + 1, :], func=Ln
                )
                rec = small.tile([D + 1, S], f32, tag="rec")
                nc.scalar.activation(
                    out=rec[D : D + 1, :], in_=lnt[D : D + 1, :], func=Exp, scale=-1.0
                )
                bc = psum.tile([P, S], f32, tag="ps")
                for nch in range(2):
                    nc.tensor.matmul(
                        bc[:, nch * 512 : (nch + 1) * 512],
                        lhsT=ones_sb[D : D + 1, :],
                        rhs=rec[D : D + 1, nch * 512 : (nch + 1) * 512],
                        start=True,
                        stop=True,
                    )
                rbc = small.tile([P, S], bf16, tag="rbc")
                nc.vector.tensor_copy(out=rbc, in_=bc)
                for kt in range(KT):
                    nc.vector.tensor_mul(
                        out=expT[:, kt, :], in0=expT[:, kt, :], in1=rbc
                    )
                nc.sync.dma_start(
                    out=attnT[h].rearrange("(t p) q -> p t q", p=P), in_=expT
                )
                # normalized ctx^T into the stacked rhs for the out-projection
                if h % 2 == 0:
                    nc.vector.tensor_mul(
                        out=ctxn_sb[0:D, h // 2, :], in0=ctx[0:D, :], in1=rbc[0:D, :]
                    )
                else:
                    stg = small.tile([D, S], bf16, tag="stg")
                    nc.vector.tensor_mul(out=stg, in0=ctx[0:D, :], in1=rbc[0:D, :])
                    nc.sync.dma_start(out=ctxn_sb[64:128, h // 2, :], in_=stg)

            # ---- output projection: outT = woT.T @ ctxn ----
            outT_r = outT.rearrange("(t p) q -> p t q", p=P)
            for mt in range(KT):
                for nch in range(2):
                    ps = psum.tile([P, S], f32, tag="ps")
                    pslice = ps[:, :512]
                    for ct in range(OT):
                        nc.tensor.matmul(
                            pslice,
                            lhsT=wo_sb[:, ct, mt * 128 : (mt + 1) * 128],
                            rhs=ctxn_sb[:, ct, nch * 512 : (nch + 1) * 512],
                            start=(ct == 0),
                            stop=(ct == OT - 1),
                        )
                    ob = outsb.tile([P, 512], f32, tag="ob")
                    nc.scalar.copy(out=ob, in_=pslice)
                    nc.sync.dma_start(
                        out=outT_r[:, mt, nch * 512 : (nch + 1) * 512], in_=ob
                    )

    nc.compile()
    return nc


def _get_nc():
    if "nc" not in _CACHE:
        _CACHE["nc"] = _build_module()
    return _CACHE["nc"]


def _make_in_maps(query, key, value, Wq, Wk, Wv, Wo):
    query, key, value, Wq, Wk, Wv, Wo = (
        np.asarray(x, dtype=np.float32) for x in (query, key, value, Wq, Wk, Wv, Wo)
    )
    in_maps = []
    for c in range(NCORES):
        b, half = divmod(c, 2)
        hs = slice(half * DH, (half + 1) * DH)
        in_maps.append(
            {
                "qT": query[b].T.astype(BF16NP),
                "kT": key[b].T.astype(BF16NP),
                "vT": value[b].T.astype(BF16NP),
                "wqT": Wq[hs, :].T.astype(BF16NP),
                "wkT": Wk[hs, :].T.astype(BF16NP),
                "wvT": Wv[hs, :].T.astype(BF16NP),
                "woT": Wo[:, hs].T.astype(BF16NP),
            }
        )
    return in_maps


def _assemble(results):
    attn = np.empty((B, H, S, S), np.float32)
    out = np.empty((B, S, DM), np.float32)
    for b in range(B):
        r0, r1 = results[2 * b], results[2 * b + 1]
        # attnT is [h, k, q]; reference wants [h, q, k]
        attn[b, :NH] = np.swapaxes(np.asarray(r0["attnT"]), 1, 2).astype(np.float32)
        attn[b, NH:] = np.swapaxes(np.asarray(r1["attnT"]), 1, 2).astype(np.float32)
        out[b] = (np.asarray(r0["outT"]) + np.asarray(r1["outT"])).T
    return out, attn


def run(trace=False, **inputs):
    from concourse import bass_utils

    nc = _get_nc()
    in_maps = _make_in_maps(**inputs)
    res = bass_utils.run_bass_kernel_spmd(
        nc, in_maps, core_ids=list(range(NCORES)), trace=trace
    )
    _CACHE["last_result"] = res
    out, attn = _assemble(res.results)
    return out, attn


def kernel(query, key, value, Wq, Wk, Wv, Wo):
    return run(
        query=query, key=key, value=value, Wq=Wq, Wk=Wk, Wv=Wv, Wo=Wo
    )


# revision 10
# speedup vs baseline: 1.2065x; 1.2065x over previous
# Multi-head attention (B=4, S=1024, H=16, D=64) on 8 trn2 NeuronCores.
#
# Sharding: core c handles batch b=c//2 and heads [8*(c%2), 8*(c%2)+8).
# Each core computes Q/K/V projections for its 512 head-dims over its
# batch's full sequence, per-head attention (scores^T layout: [k, q] so
# softmax sums come from a ones-column packed into the ctx matmul), and
# a partial output projection; the host sums the two half-head partials
# per batch and view-transposes the [k, q] attention back to [q, k].
#
# All device compute is bf16 on the PE with f32 PSUM accumulation; the
# attention output is written bf16 and upcast on the host.

import os
import sys

for _p in ("/opt/trn_rl_repo", "/root/.axon_site/_ro/trn_rl_repo"):
    if os.path.isdir(_p) and _p not in sys.path:
        sys.path.append(_p)

import numpy as np
import ml_dtypes

B, S, H, D = 4, 1024, 16, 64
DM = 1024      # model dim
NH = 8         # heads per core
DH = NH * D    # 512 head-dims per core
P = 128
KT = DM // P   # 8 contraction / seq tiles
OT = DH // P   # 4 out-dim tiles per core
NCORES = 8
BF16NP = ml_dtypes.bfloat16

_CACHE = {}


def _build_module():
    import concourse.tile as tile
    from concourse import bacc, mybir

    f32 = mybir.dt.float32
    bf16 = mybir.dt.bfloat16
    Exp = mybir.ActivationFunctionType.Exp
    Ln = mybir.ActivationFunctionType.Ln

    nc = bacc.Bacc(
        "TRN2", target_bir_lowering=False, debug=False, num_devices=NCORES
    )

    qT = nc.dram_tensor("qT", [DM, S], bf16, kind="ExternalInput").ap()
    kT = nc.dram_tensor("kT", [DM, S], bf16, kind="ExternalInput").ap()
    vT = nc.dram_tensor("vT", [DM, S], bf16, kind="ExternalInput").ap()
    wqT = nc.dram_tensor("wqT", [DM, DH], bf16, kind="ExternalInput").ap()
    wkT = nc.dram_tensor("wkT", [DM, DH], bf16, kind="ExternalInput").ap()
    wvT = nc.dram_tensor("wvT", [DM, DH], bf16, kind="ExternalInput").ap()
    woT = nc.dram_tensor("woT", [DH, DM], bf16, kind="ExternalInput").ap()
    attnT = nc.dram_tensor("attnT", [NH, S, S], bf16, kind="ExternalOutput").ap()
    outT = nc.dram_tensor("outT", [DM, S], f32, kind="ExternalOutput").ap()

    with tile.TileContext(nc) as tc:
        with (
            tc.tile_pool(name="singles", bufs=1) as singles,
            tc.tile_pool(name="exps", bufs=3) as exps,
            tc.tile_pool(name="small", bufs=2) as small,
            tc.tile_pool(name="outsb", bufs=3) as outsb,
            tc.tile_pool(name="ps", bufs=2, space="PSUM") as psum,
            tc.tile_pool(name="ctxp", bufs=2, space="PSUM") as ctxpsum,
        ):
            # ---- stage inputs in SBUF ----
            qT_sb = singles.tile([P, KT, S], bf16)
            kT_sb = singles.tile([P, KT, S], bf16)
            vT_sb = singles.tile([P, KT, S], bf16)
            nc.sync.dma_start(out=qT_sb, in_=qT.rearrange("(t p) s -> p t s", p=P))
            nc.sync.dma_start(out=kT_sb, in_=kT.rearrange("(t p) s -> p t s", p=P))
            nc.sync.dma_start(out=vT_sb, in_=vT.rearrange("(t p) s -> p t s", p=P))
            wq_sb = singles.tile([P, KT, DH], bf16)
            wk_sb = singles.tile([P, KT, DH], bf16)
            wv_sb = singles.tile([P, KT, DH], bf16)
            nc.sync.dma_start(out=wq_sb, in_=wqT.rearrange("(t p) d -> p t d", p=P))
            nc.sync.dma_start(out=wk_sb, in_=wkT.rearrange("(t p) d -> p t d", p=P))
            nc.sync.dma_start(out=wv_sb, in_=wvT.rearrange("(t p) d -> p t d", p=P))
            wo_sb = singles.tile([P, OT, DM], bf16)
            nc.sync.dma_start(out=wo_sb, in_=woT.rearrange("(t p) d -> p t d", p=P))


            # QpT/KpT: [part = head-dim within tile, ot, seq]; head h lives on
            # partitions 64*(h%2).. of tile h//2.
            qp_sb = singles.tile([P, OT, S], bf16)
            kp_sb = singles.tile([P, OT, S], bf16)
            # Vp: [part = seq within tile, seq-tile, head, d + ones column]
            vp_sb = singles.tile([P, KT, NH, D + 1], bf16)
            # normalized ctx^T stacked: [part = head-dim within tile, ct, q]
            ctxn_sb = singles.tile([P, OT, S], bf16)

            # ---- Q/K projections: QpT = wqT.T @ qT ----
            for w_sb, x_sb, dst in ((wq_sb, qT_sb, qp_sb), (wk_sb, kT_sb, kp_sb)):
                for ot in range(OT):
                    for nch in range(2):
                        ps = psum.tile([P, S], f32, tag="ps")
                        pslice = ps[:, :512]
                        for kt in range(KT):
                            nc.tensor.matmul(
                                pslice,
                                lhsT=w_sb[:, kt, ot * 128 : (ot + 1) * 128],
                                rhs=x_sb[:, kt, nch * 512 : (nch + 1) * 512],
                                start=(kt == 0),
                                stop=(kt == KT - 1),
                            )
                        nc.vector.tensor_copy(
                            out=dst[:, ot, nch * 512 : (nch + 1) * 512], in_=pslice
                        )

            # ---- V projection: Vp = vT.T.T ... Vp[seq, dh] = (vT.T) @ wvT ----
            nc.vector.memset(vp_sb[:, :, :, D], 1.0)
            for mt in range(KT):
                ps = psum.tile([P, S], f32, tag="ps")
                pslice = ps[:, :512]
                for kt in range(KT):
                    nc.tensor.matmul(
                        pslice,
                        lhsT=vT_sb[:, kt, mt * 128 : (mt + 1) * 128],
                        rhs=wv_sb[:, kt, :],
                        start=(kt == 0),
                        stop=(kt == KT - 1),
                    )
                nc.vector.tensor_copy(
                    out=vp_sb[:, mt, :, 0:D],
                    in_=pslice.rearrange("p (h d) -> p h d", h=NH),
                )

            # ---- attention, software-pipelined: scores/exp of head h
            # interleave with the ctx matmuls of head h-1 so the PE stays
            # busy while ScalarE runs exp ----
            expTs = {}
            ctxs = {}
            for c in range(NH + 1):
                if c < NH:
                    expTs[c] = exps.tile([P, KT, S], bf16, tag="expT", name=f"expT{c}")
                    ctxs[c] = ctxpsum.tile([P, S], f32, tag="ctx", name=f"ctx{c}")
                for kt in range(KT):
                    if c < NH:
                        hp = 64 * (c % 2)
                        ot = c // 2
                        sc = psum.tile([P, S], f32, tag="ps")
                        for nch in range(2):
                            nc.tensor.matmul(
                                sc[:, nch * 512 : (nch + 1) * 512],
                                lhsT=kp_sb[
                                    hp : hp + 64, ot, kt * 128 : (kt + 1) * 128
                                ],
                                rhs=qp_sb[hp : hp + 64, ot, nch * 512 : (nch + 1) * 512],
                                start=True,
                                stop=True,
                            )
                        # exp(scores/8); bf16 out feeds both the ctx matmul
                        # and, after normalization, the attn output
                        nc.scalar.activation(
                            out=expTs[c][:, kt, :], in_=sc[:], func=Exp, scale=1.0 / 8.0
                        )
                    if c >= 1:
                        t = c - 1
                        nc_expT = expTs[t]
                        for nch in range(2):
                            nc.tensor.matmul(
                                ctxs[t][0 : D + 1, nch * 512 : (nch + 1) * 512],
                                lhsT=vp_sb[:, kt, t, :],
                                rhs=nc_expT[:, kt, nch * 512 : (nch + 1) * 512],
                                start=(kt == 0),
                                stop=(kt == KT - 1),
                            )
                if c >= 1:
                    # tail for head t = c-1: normalizer + attn + ctxn
                    t = c - 1
                    ctx = ctxs.pop(t)
                    expT = expTs.pop(t)
                    # 1/s = exp(-ln(s)) on ScalarE (custom DVE recip is
                    # broken on HW at base partition != 0); shift the row
                    # to base 0 so gpsimd partition_broadcast can read it
                    lnt = small.tile([1, S], f32, tag="lnt")
                    nc.scalar.activation(
                        out=lnt, in_=ctx[D : D + 1, :], func=Ln
                    )
                    rec = small.tile([1, S], bf16, tag="rec")
                    nc.scalar.activation(out=rec, in_=lnt, func=Exp, scale=-1.0)
                    rbc = small.tile([P, S], bf16, tag="rbc")
                    nc.gpsimd.partition_broadcast(out_ap=rbc, in_ap=rec)
                    for kt in range(KT):
                        nc.vector.tensor_mul(
                            out=expT[:, kt, :], in0=expT[:, kt, :], in1=rbc
                        )
                    nc.sync.dma_start(
                        out=attnT[t].rearrange("(t p) q -> p t q", p=P), in_=expT
                    )
                    # normalized ctx^T into the stacked rhs for the
                    # out-projection (DVE handles the partition-base shift)
                    base = 64 * (t % 2)
                    nc.vector.tensor_mul(
                        out=ctxn_sb[base : base + 64, t // 2, :],
                        in0=ctx[0:D, :],
                        in1=rbc[0:D, :],
                    )

            # ---- output projection: outT = woT.T @ ctxn ----
            outT_r = outT.rearrange("(t p) q -> p t q", p=P)
            for mt in range(KT):
                for nch in range(2):
                    ps = psum.tile([P, S], f32, tag="ps")
                    pslice = ps[:, :512]
                    for ct in range(OT):
                        nc.tensor.matmul(
                            pslice,
                            lhsT=wo_sb[:, ct, mt * 128 : (mt + 1) * 128],
                            rhs=ctxn_sb[:, ct, nch * 512 : (nch + 1) * 512],
                            start=(ct == 0),
                            stop=(ct == OT - 1),
                        )
                    ob = outsb.tile([P, 512], f32, tag="ob")
                    nc.vector.tensor_copy(out=ob, in_=pslice)
                    nc.sync.dma_start(
                        out=outT_r[:, mt, nch * 512 : (nch + 1) * 512], in_=ob
                    )

    nc.compile()
    return nc


def _get_nc():
    if "nc" not in _CACHE:
        _CACHE["nc"] = _build_module()
    return _CACHE["nc"]


def _make_in_maps(query, key, value, Wq, Wk, Wv, Wo):
    query, key, value, Wq, Wk, Wv, Wo = (
        np.asarray(x, dtype=np.float32) for x in (query, key, value, Wq, Wk, Wv, Wo)
    )
    in_maps = []
    for c in range(NCORES):
        b, half = divmod(c, 2)
        hs = slice(half * DH, (half + 1) * DH)
        in_maps.append(
            {
                "qT": query[b].T.astype(BF16NP),
                "kT": key[b].T.astype(BF16NP),
                "vT": value[b].T.astype(BF16NP),
                "wqT": Wq[hs, :].T.astype(BF16NP),
                "wkT": Wk[hs, :].T.astype(BF16NP),
                "wvT": Wv[hs, :].T.astype(BF16NP),
                "woT": Wo[:, hs].T.astype(BF16NP),
            }
        )
    return in_maps


def _assemble(results):
    attn = np.empty((B, H, S, S), np.float32)
    out = np.empty((B, S, DM), np.float32)
    for b in range(B):
        r0, r1 = results[2 * b], results[2 * b + 1]
        # attnT is [h, k, q]; reference wants [h, q, k]
        attn[b, :NH] = np.swapaxes(np.asarray(r0["attnT"]), 1, 2).astype(np.float32)
        attn[b, NH:] = np.swapaxes(np.asarray(r1["attnT"]), 1, 2).astype(np.float32)
        out[b] = (np.asarray(r0["outT"]) + np.asarray(r1["outT"])).T
    return out, attn


def run(trace=False, **inputs):
    from concourse import bass_utils

    nc = _get_nc()
    in_maps = _make_in_maps(**inputs)
    res = bass_utils.run_bass_kernel_spmd(
        nc, in_maps, core_ids=list(range(NCORES)), trace=trace
    )
    _CACHE["last_result"] = res
    out, attn = _assemble(res.results)
    return out, attn


def kernel(query, key, value, Wq, Wk, Wv, Wo):
    return run(
        query=query, key=key, value=value, Wq=Wq, Wk=Wk, Wv=Wv, Wo=Wo
    )


# revision 12
# speedup vs baseline: 1.3138x; 1.0890x over previous
# Multi-head attention (B=4, S=1024, H=16, D=64) on 8 trn2 NeuronCores.
#
# Sharding: core c handles batch b=c//2 and heads [8*(c%2), 8*(c%2)+8).
# Each core computes Q/K/V projections for its 512 head-dims over its
# batch's full sequence, per-head attention (scores^T layout: [k, q] so
# softmax sums come from a ones-column packed into the ctx matmul), and
# a partial output projection; the host sums the two half-head partials
# per batch and view-transposes the [k, q] attention back to [q, k].
#
# All device compute is bf16 on the PE with f32 PSUM accumulation; the
# attention output is written bf16 and upcast on the host.

import os
import sys

for _p in ("/opt/trn_rl_repo", "/root/.axon_site/_ro/trn_rl_repo"):
    if os.path.isdir(_p) and _p not in sys.path:
        sys.path.append(_p)

import numpy as np
import ml_dtypes

B, S, H, D = 4, 1024, 16, 64
DM = 1024      # model dim
NH = 8         # heads per core
DH = NH * D    # 512 head-dims per core
P = 128
KT = DM // P   # 8 contraction / seq tiles
OT = DH // P   # 4 out-dim tiles per core
NCORES = 8
BF16NP = ml_dtypes.bfloat16

_CACHE = {}


def _build_module():
    import concourse.tile as tile
    from concourse import bacc, mybir

    f32 = mybir.dt.float32
    bf16 = mybir.dt.bfloat16
    Exp = mybir.ActivationFunctionType.Exp
    Ln = mybir.ActivationFunctionType.Ln

    nc = bacc.Bacc(
        "TRN2", target_bir_lowering=False, debug=False, num_devices=NCORES
    )

    qT = nc.dram_tensor("qT", [DM, S], bf16, kind="ExternalInput").ap()
    kT = nc.dram_tensor("kT", [DM, S], bf16, kind="ExternalInput").ap()
    vT = nc.dram_tensor("vT", [DM, S], bf16, kind="ExternalInput").ap()
    wqT = nc.dram_tensor("wqT", [DM, DH], bf16, kind="ExternalInput").ap()
    wkT = nc.dram_tensor("wkT", [DM, DH], bf16, kind="ExternalInput").ap()
    wvT = nc.dram_tensor("wvT", [DM, DH], bf16, kind="ExternalInput").ap()
    woT = nc.dram_tensor("woT", [DH, DM], bf16, kind="ExternalInput").ap()
    # outputs in partition-major layouts so DMA runs are 16KB contiguous:
    # attnT[h, p, t, q] with k = t*128+p ; outT[p, mt, q] with dm = mt*128+p
    attnT = nc.dram_tensor(
        "attnT", [NH, P, KT, S], bf16, kind="ExternalOutput"
    ).ap()
    outT = nc.dram_tensor("outT", [P, KT, S], f32, kind="ExternalOutput").ap()

    with tile.TileContext(nc) as tc:
        with (
            tc.tile_pool(name="singles", bufs=1) as singles,
            tc.tile_pool(name="exps", bufs=3) as exps,
            tc.tile_pool(name="small", bufs=2) as small,
            tc.tile_pool(name="outsb", bufs=3) as outsb,
            tc.tile_pool(name="ps", bufs=2, space="PSUM") as psum,
            tc.tile_pool(name="ctxp", bufs=2, space="PSUM") as ctxpsum,
        ):
            # ---- stage inputs in SBUF ----
            qT_sb = singles.tile([P, KT, S], bf16)
            kT_sb = singles.tile([P, KT, S], bf16)
            vT_sb = singles.tile([P, KT, S], bf16)
            wq_sb = singles.tile([P, KT, DH], bf16)
            wk_sb = singles.tile([P, KT, DH], bf16)
            wv_sb = singles.tile([P, KT, DH], bf16)
            wo_sb = singles.tile([P, OT, DM], bf16)
            # host supplies partition-major [(p t), cols]; load per k-tile so
            # the projections can start before the full tensors land
            loads = [
                (qT_sb, qT, S), (wq_sb, wqT, DH),
                (kT_sb, kT, S), (wk_sb, wkT, DH),
                (vT_sb, vT, S), (wv_sb, wvT, DH),
            ]
            for kt in range(KT):
                for sb_t, dram_t, _cols in loads:
                    nc.sync.dma_start(
                        out=sb_t[:, kt, :],
                        in_=dram_t.rearrange("(p t) c -> p t c", p=P)[:, kt, :],
                    )
            nc.sync.dma_start(out=wo_sb, in_=woT.rearrange("(p t) d -> p t d", p=P))


            # QpT/KpT: [part = head-dim within tile, ot, seq]; head h lives on
            # partitions 64*(h%2).. of tile h//2.
            qp_sb = singles.tile([P, OT, S], bf16)
            kp_sb = singles.tile([P, OT, S], bf16)
            # Vp: [part = seq within tile, seq-tile, head, d + ones column]
            vp_sb = singles.tile([P, KT, NH, D + 1], bf16)
            # normalized ctx^T stacked: [part = head-dim within tile, ct, q]
            ctxn_sb = singles.tile([P, OT, S], bf16)

            # ---- Q/K projections: QpT = wqT.T @ qT ----
            for w_sb, x_sb, dst in ((wq_sb, qT_sb, qp_sb), (wk_sb, kT_sb, kp_sb)):
                for ot in range(OT):
                    for nch in range(2):
                        ps = psum.tile([P, S], f32, tag="ps")
                        pslice = ps[:, :512]
                        for kt in range(KT):
                            nc.tensor.matmul(
                                pslice,
                                lhsT=w_sb[:, kt, ot * 128 : (ot + 1) * 128],
                                rhs=x_sb[:, kt, nch * 512 : (nch + 1) * 512],
                                start=(kt == 0),
                                stop=(kt == KT - 1),
                            )
                        nc.vector.tensor_copy(
                            out=dst[:, ot, nch * 512 : (nch + 1) * 512], in_=pslice
                        )

            # ---- V projection: Vp = vT.T.T ... Vp[seq, dh] = (vT.T) @ wvT ----
            nc.vector.memset(vp_sb[:, :, :, D], 1.0)
            for mt in range(KT):
                ps = psum.tile([P, S], f32, tag="ps")
                pslice = ps[:, :512]
                for kt in range(KT):
                    nc.tensor.matmul(
                        pslice,
                        lhsT=vT_sb[:, kt, mt * 128 : (mt + 1) * 128],
                        rhs=wv_sb[:, kt, :],
                        start=(kt == 0),
                        stop=(kt == KT - 1),
                    )
                nc.vector.tensor_copy(
                    out=vp_sb[:, mt, :, 0:D],
                    in_=pslice.rearrange("p (h d) -> p h d", h=NH),
                )

            # ---- attention, software-pipelined: scores/exp of head h
            # interleave with the ctx matmuls of head h-1 so the PE stays
            # busy while ScalarE runs exp ----
            expTs = {}
            ctxs = {}
            for c in range(NH + 1):
                if c < NH:
                    expTs[c] = exps.tile([P, KT, S], bf16, tag="expT", name=f"expT{c}")
                    ctxs[c] = ctxpsum.tile([P, S], f32, tag="ctx", name=f"ctx{c}")
                for kt in range(KT):
                    if c < NH:
                        hp = 64 * (c % 2)
                        ot = c // 2
                        sc = psum.tile([P, S], f32, tag="ps")
                        for nch in range(2):
                            nc.tensor.matmul(
                                sc[:, nch * 512 : (nch + 1) * 512],
                                lhsT=kp_sb[
                                    hp : hp + 64, ot, kt * 128 : (kt + 1) * 128
                                ],
                                rhs=qp_sb[hp : hp + 64, ot, nch * 512 : (nch + 1) * 512],
                                start=True,
                                stop=True,
                            )
                        # exp(scores/8); bf16 out feeds both the ctx matmul
                        # and, after normalization, the attn output
                        nc.scalar.activation(
                            out=expTs[c][:, kt, :], in_=sc[:], func=Exp, scale=1.0 / 8.0
                        )
                    if c >= 1:
                        t = c - 1
                        nc_expT = expTs[t]
                        for nch in range(2):
                            nc.tensor.matmul(
                                ctxs[t][0 : D + 1, nch * 512 : (nch + 1) * 512],
                                lhsT=vp_sb[:, kt, t, :],
                                rhs=nc_expT[:, kt, nch * 512 : (nch + 1) * 512],
                                start=(kt == 0),
                                stop=(kt == KT - 1),
                            )
                if c >= 1:
                    # tail for head t = c-1: normalizer + ctxn + attn
                    t = c - 1
                    ctx = ctxs.pop(t)
                    expT = expTs.pop(t)
                    # shift the sums row to base partition 0 (engines can
                    # retarget partition bases), then the fast custom-DVE
                    # reciprocal (only correct at base 0 on HW)
                    sums = small.tile([1, S], f32, tag="sums")
                    nc.vector.tensor_copy(out=sums, in_=ctx[D : D + 1, :])
                    recf = small.tile([1, S], f32, tag="recf")
                    nc.vector.reciprocal_approx_fast(out=recf, in_=sums)
                    rec = small.tile([1, S], bf16, tag="rec")
                    nc.vector.tensor_copy(out=rec, in_=recf)
                    rbc = small.tile([P, S], bf16, tag="rbc")
                    nc.gpsimd.partition_broadcast(out_ap=rbc, in_ap=rec)
                    # ctxn first: it gates the output projection
                    base = 64 * (t % 2)
                    nc.vector.tensor_mul(
                        out=ctxn_sb[base : base + 64, t // 2, :],
                        in0=ctx[0:D, :],
                        in1=rbc[0:D, :],
                    )
                    # one 3D op for the whole head's attn normalization
                    nc.vector.tensor_mul(
                        out=expT,
                        in0=expT,
                        in1=rbc.unsqueeze(1).to_broadcast((P, KT, S)),
                    )
                    nc.sync.dma_start(out=attnT[t], in_=expT)

            # ---- output projection: outT = woT.T @ ctxn ----
            for mt in range(KT):
                ob = outsb.tile([P, S], f32, tag="ob")
                for nch in range(2):
                    ps = psum.tile([P, S], f32, tag="ps")
                    pslice = ps[:, :512]
                    for ct in range(OT):
                        nc.tensor.matmul(
                            pslice,
                            lhsT=wo_sb[:, ct, mt * 128 : (mt + 1) * 128],
                            rhs=ctxn_sb[:, ct, nch * 512 : (nch + 1) * 512],
                            start=(ct == 0),
                            stop=(ct == OT - 1),
                        )
                    nc.scalar.copy(
                        out=ob[:, nch * 512 : (nch + 1) * 512], in_=pslice
                    )
                nc.sync.dma_start(out=outT[:, mt, :], in_=ob)

    nc.compile()
    return nc


def _get_nc():
    if "nc" not in _CACHE:
        _CACHE["nc"] = _build_module()
    return _CACHE["nc"]


def _make_in_maps(query, key, value, Wq, Wk, Wv, Wo):
    query, key, value, Wq, Wk, Wv, Wo = (
        np.asarray(x, dtype=np.float32) for x in (query, key, value, Wq, Wk, Wv, Wo)
    )
    in_maps = []
    for c in range(NCORES):
        b, half = divmod(c, 2)
        hs = slice(half * DH, (half + 1) * DH)
        def pmajor(arr2d):
            # [(t p), c] -> [(p t), c] so each SBUF partition's data is one
            # contiguous DRAM run
            r, cdim = arr2d.shape
            t = r // P
            return (
                arr2d.reshape(t, P, cdim).swapaxes(0, 1).reshape(r, cdim)
            )

        in_maps.append(
            {
                "qT": pmajor(query[b].T.astype(BF16NP)),
                "kT": pmajor(key[b].T.astype(BF16NP)),
                "vT": pmajor(value[b].T.astype(BF16NP)),
                "wqT": pmajor(Wq[hs, :].T.astype(BF16NP)),
                "wkT": pmajor(Wk[hs, :].T.astype(BF16NP)),
                "wvT": pmajor(Wv[hs, :].T.astype(BF16NP)),
                "woT": pmajor(Wo[:, hs].T.astype(BF16NP)),
            }
        )
    return in_maps


def _assemble(results):
    attn = np.empty((B, H, S, S), np.float32)
    out = np.empty((B, S, DM), np.float32)
    for b in range(B):
        r0, r1 = results[2 * b], results[2 * b + 1]
        # attnT is [h, p, t, q] with k = t*128+p; reference wants [h, q, k]
        for half, r in ((0, r0), (1, r1)):
            a = np.asarray(r["attnT"])  # [8, 128, 8, 1024]
            a = a.transpose(0, 3, 2, 1).reshape(NH, S, S)  # [h, q, (t p)=k]
            attn[b, half * NH : (half + 1) * NH] = a.astype(np.float32)
        # outT is [p, mt, q] with dm = mt*128+p
        o = np.asarray(r0["outT"]) + np.asarray(r1["outT"])
        out[b] = o.transpose(2, 1, 0).reshape(S, DM)
    return out, attn


def run(trace=False, **inputs):
    from concourse import bass_utils

    nc = _get_nc()
    in_maps = _make_in_maps(**inputs)
    res = bass_utils.run_bass_kernel_spmd(
        nc, in_maps, core_ids=list(range(NCORES)), trace=trace
    )
    _CACHE["last_result"] = res
    out, attn = _assemble(res.results)
    return out, attn


def kernel(query, key, value, Wq, Wk, Wv, Wo):
    return run(
        query=query, key=key, value=value, Wq=Wq, Wk=Wk, Wv=Wv, Wo=Wo
    )
